# revision 20
# baseline (speedup 1.0000x reference)
"""EquivariantLayer GNN message passing on 8 Trainium2 NeuronCores.

Strategy (node-parallel, folded weights, v5):
- Per-edge attention folds to quadratic forms in rel (6 monomials); softmax
  is taken relative to head 0 (3 delta-heads, exp(0)=1).
- The 6->3 score contraction runs on the PE: monomials are stored
  slot-interleaved [P, W, 8] (2 pad channels), DMA-transposed in 128-column
  blocks, and multiplied by a block-diagonal Cd matrix; exp reads the
  scores straight out of PSUM on the scalar engine.
- The edge axis is cut into 4 tier-aligned, 16-slot-aligned slices, each
  flowing load -> rel -> monomials -> transpose -> PE scores -> exp ->
  softmax -> F products -> slot trees independently (software pipeline).
- Counts cancel inside LayerNorm: LN(s/n) = LN(s); only a per-node
  32*n^2*eps correction enters the variance. G is row-centered on host so
  the matmul emits centered values directly.
- Dummy edge slots carry the destination position so rel == 0 exactly:
  no mask needed; counts are host-precomputed.
- bf16 on DVE (2x modes) everywhere except f32 score accumulation (PE) and
  the variance; activation-table switches are grouped (exp_and_others
  covers Square/Exp/Copy; Sqrt then Silu each load once).
- The device also emits the per-node raw variance; the host recomputes the
  rare ill-conditioned nodes (tiny LN variance amplifies rounding) exactly.
- DMA issue order matches dependency order (the SP queue is in-order).
"""
import numpy as np

N_NODES = 100000
N_EDGES = 500000
HIDDEN = 32
HEADS = 4
LN_EPS = 1e-5
N_CORES = 8

P = 128
NPC = N_NODES // N_CORES          # 12500 nodes per core
# degree tiers: (max_degree_in_tier, node-locs per partition); boundaries
# are multiples of 16 slots so transpose blocks align with tiers
TIERS = [(2, 16), (4, 32), (6, 32), (8, 18), (10, 8), (12, 4), (18, 2)]
T_D = [t[0] for t in TIERS]
T_LOC = [t[1] for t in TIERS]
T_W = [d * l for d, l in TIERS]
T_W0 = np.concatenate([[0], np.cumsum(T_W)]).tolist()   # ...660
T_L0 = np.concatenate([[0], np.cumsum(T_LOC)]).tolist()
W = 672                           # 660 used + 12 dead cols, multiple of 16
NL = sum(T_LOC)                   # 112 node-locs per partition
NLP = 112
NBLK = NLP // 8                   # 14
# pipeline slices (tier ranges); w-spans are multiples of 16
SLICES = [(0, 2), (2, 3), (3, 4), (4, 7)]
SL_W = [(T_W0[a], T_W0[b] if b < 7 else W) for a, b in SLICES]
LN_SPLITS = [(0, 48), (48, 96), (96, NL)]  # row-split LayerNorm tail
VAR_TAU = 1e-3                    # host-fixup threshold on LN variance


def _fold_weights(Wq, bq, Wk, bk, Wv, bv, Wout):
    s = 1.0 / np.sqrt(np.float32(HIDDEN))
    C = np.zeros((6, HEADS), np.float32)
    D = HIDDEN
    for h in range(HEADS):
        Wqh, Wkh = Wq[:, h * D:(h + 1) * D], Wk[:, h * D:(h + 1) * D]
        A = (Wqh @ Wkh.T) * s
        C[0, h] = A[0, 0]; C[1, h] = A[0, 1] + A[1, 0]; C[2, h] = A[0, 2] + A[2, 0]
        C[3, h] = A[1, 1]; C[4, h] = A[1, 2] + A[2, 1]; C[5, h] = A[2, 2]
    Cd = C[:, 1:] - C[:, 0:1]     # delta-scores vs head 0
    G16 = np.zeros((16, 32), np.float32)
    for h in range(HEADS):
        Wvh, bvh = Wv[:, h * D:(h + 1) * D], bv[h * D:(h + 1) * D]
        Wouth = Wout[h * D:(h + 1) * D, :]
        Gh = Wvh @ Wouth
        for d in range(3):
            G16[3 * h + d, :] = Gh[d]
        G16[12 + h, :] = bvh @ Wouth
    return Cd, G16


def _tree_reduce(nc, Alu, F12, Fagg, NCH, ti):
    """Slot-sum for one tier via in-place halving adds on F12."""
    d, l, tw0, tl0 = T_D[ti], T_LOC[ti], T_W0[ti], T_L0[ti]
    fv = F12[:, :, tw0:tw0 + d * l].rearrange("p j (n s) -> p j n s", s=d)
    cur = d
    while cur > 2:
        if cur % 2:
            nc.vector.tensor_tensor(
                out=fv[:, :, :, 0:1], in0=fv[:, :, :, 0:1],
                in1=fv[:, :, :, cur - 1:cur], op=Alu.add)
            cur -= 1
        half = cur // 2
        nc.vector.tensor_tensor(
            out=fv[:, :, :, :half], in0=fv[:, :, :, :half],
            in1=fv[:, :, :, half:cur], op=Alu.add)
        cur = half
    out = Fagg[:, tl0:tl0 + l, :NCH].rearrange("p n j -> p j n")
    if cur == 2:
        nc.vector.tensor_tensor(out=out, in0=fv[:, :, :, 0],
                                in1=fv[:, :, :, 1], op=Alu.add)
    else:
        nc.vector.tensor_copy(out=out, in_=fv[:, :, :, 0])


def _build_bass(use_bout, use_affine, use_gbias):
    import concourse.bass as bass
    import concourse.bacc as bacc
    import concourse.mybir as mybir
    import concourse.tile as tile

    f32 = mybir.dt.float32
    bf16 = mybir.dt.bfloat16
    Alu = mybir.AluOpType
    Act = mybir.ActivationFunctionType
    NCH = 16 if use_gbias else 12   # F channels fed through the tree

    nc = bacc.Bacc("TRN2", target_bir_lowering=False, debug=False,
                   num_devices=N_CORES)
    A_in = nc.dram_tensor("A", [P, 3, W], f32, kind="ExternalInput").ap()
    B_in = nc.dram_tensor("B", [P, 5, NL], f32, kind="ExternalInput").ap()
    G_in = nc.dram_tensor("G", [P, 256], bf16, kind="ExternalInput").ap()
    CD_in = nc.dram_tensor("CD", [P, 48], bf16, kind="ExternalInput").ap()
    AUX_in = nc.dram_tensor("AUX", [P, 3, 32], f32, kind="ExternalInput").ap()
    y = nc.dram_tensor("y", [P, 32, NL], bf16, kind="ExternalOutput").ap()
    vr = nc.dram_tensor("vr", [P, NL], f32, kind="ExternalOutput").ap()

    with tile.TileContext(nc) as tc:
        with (
            tc.tile_pool(name="sbuf", bufs=1) as sb,
            tc.tile_pool(name="psum", bufs=4, space="PSUM") as ps,
        ):
            A = sb.tile([P, 3, W], f32)
            B = sb.tile([P, 5, NL], f32)
            G = sb.tile([P, 256], bf16)
            CD = sb.tile([P, 48], bf16)
            relb = sb.tile([P, 3, W], bf16)
            M6 = sb.tile([P, W, 8], bf16)
            M6T = sb.tile([P, W // 16, P], bf16)
            E = sb.tile([P, 3, W], bf16)
            den = sb.tile([P, W], bf16)
            rinv = sb.tile([P, W], bf16)
            ATT = sb.tile([P, 4, W], bf16)
            F12 = sb.tile([P, NCH, W], bf16)
            Fagg = sb.tile([P, NLP, 16], bf16)
            FaggT = sb.tile([P, NBLK, P], bf16)
            Seg = sb.tile([P, 32, NLP], bf16)   # channel-major (packed rows)
            sq = sb.tile([P, 32, NL], bf16)
            var = sb.tile([P, NL], f32)
            std = sb.tile([P, NL], f32)
            rstd = sb.tile([P, NL], bf16)
            X = sb.tile([P, 32, NL], bf16)
            Y = sb.tile([P, 32, NL], bf16)
            AUX = sb.tile([P, 3, 32], f32)

            # loads in dependency-use order (SP queue is in-order)
            nc.sync.dma_start(out=B[:], in_=B_in[:])
            for (w0, w1) in SL_W:
                nc.sync.dma_start(out=A[:, :, w0:w1], in_=A_in[:, :, w0:w1])
            nc.sync.dma_start(out=CD[:], in_=CD_in[:])
            nc.sync.dma_start(out=G[:], in_=G_in[:])
            if use_bout or use_affine:
                nc.sync.dma_start(out=AUX[:], in_=AUX_in[:])
            # zero pad channels / dead cols before use
            nc.gpsimd.memset(M6[:, :, 6:8], 0.0)
            nc.gpsimd.memset(relb[:, :, T_W0[7]:], 0.0)
            if NCH < 16:
                nc.gpsimd.memset(Fagg[:, :, NCH:], 0.0)

            # per-slice: rel -> monomials -> blocked transpose
            for si, (ta, tb) in enumerate(SLICES):
                w0, w1 = SL_W[si]
                sl = slice(w0, w1)
                for ti in range(ta, tb):
                    d, l, tw0, tl0 = T_D[ti], T_LOC[ti], T_W0[ti], T_L0[ti]
                    nc.vector.tensor_tensor(
                        out=relb[:, :, tw0:tw0 + d * l].rearrange(
                            "p c (n s) -> p c n s", s=d),
                        in0=A[:, :, tw0:tw0 + d * l].rearrange(
                            "p c (n s) -> p c n s", s=d),
                        in1=B[:, :3, tl0:tl0 + l].unsqueeze(3).broadcast_to(
                            [P, 3, l, d]),
                        op=Alu.subtract)
                for k, i in ((0, 0), (3, 1), (5, 2)):
                    nc.scalar.activation(out=M6[:, sl, k],
                                         in_=relb[:, i, sl], func=Act.Square)
                for k, (i, j) in ((1, (0, 1)), (2, (0, 2)), (4, (1, 2))):
                    nc.vector.tensor_tensor(out=M6[:, sl, k],
                                            in0=relb[:, i, sl],
                                            in1=relb[:, j, sl], op=Alu.mult)
                nc.sync.dma_start_transpose(
                    out=M6T[:, w0 // 16:w1 // 16, :],
                    in_=M6[:, sl, :].rearrange("p w c -> p (w c)"))

            # per-slice: PE scores + exp from PSUM, softmax, F products
            for si, (ta, tb) in enumerate(SLICES):
                w0, w1 = SL_W[si]
                wc = w1 - w0
                sl = slice(w0, w1)
                blocks = list(range(w0 // 16, w1 // 16))
                for g0 in range(0, len(blocks), 8):
                    gb = blocks[g0:g0 + 8]
                    psc = ps.tile([P, 48 * len(gb)], f32, space="PSUM",
                                  tag="sc")
                    for bi, b in enumerate(gb):
                        nc.tensor.matmul(out=psc[:, 48 * bi:48 * (bi + 1)],
                                         lhsT=M6T[:, b, :], rhs=CD[:],
                                         start=True, stop=True)
                    wt0 = gb[0] * 16
                    wt1 = wt0 + 16 * len(gb)
                    nc.scalar.activation(
                        out=E[:, :, wt0:wt1],
                        in_=psc[:].rearrange("p (b s h) -> p h (b s)",
                                             h=3, s=16),
                        func=Act.Exp)
                # denominator = 1 + e1 + e2 + e3 (bf16, on the critical path
                # so kept on DVE at 2x rather than the slower Pool)
                nc.vector.tensor_tensor(out=den[:, sl], in0=E[:, 0, sl],
                                        in1=E[:, 1, sl], op=Alu.add)
                nc.vector.tensor_tensor(out=den[:, sl], in0=den[:, sl],
                                        in1=E[:, 2, sl], op=Alu.add)
                nc.vector.tensor_scalar(out=den[:, sl], in0=den[:, sl],
                                        scalar1=1.0, scalar2=None,
                                        op0=Alu.add)
                with nc.allow_low_precision(reason="bf16 softmax"):
                    nc.vector.reciprocal(out=rinv[:, sl], in_=den[:, sl])
                nc.vector.tensor_copy(out=ATT[:, 0, sl], in_=rinv[:, sl])
                nc.vector.tensor_tensor(
                    out=ATT[:, 1:4, sl], in0=E[:, :, sl],
                    in1=rinv[:, sl].unsqueeze(1).broadcast_to([P, 3, wc]),
                    op=Alu.mult)
                # F products: 12 channels (h, d) = attn_h * rel_d
                nc.vector.tensor_tensor(
                    out=F12[:, 0:9, sl].rearrange("p (h d) w -> p h d w",
                                                  d=3),
                    in0=ATT[:, 0:3, sl].unsqueeze(2).broadcast_to(
                        [P, 3, 3, wc]),
                    in1=relb[:, :, sl].unsqueeze(1).broadcast_to(
                        [P, 3, 3, wc]),
                    op=Alu.mult)
                nc.gpsimd.tensor_tensor(
                    out=F12[:, 9:12, sl],
                    in0=ATT[:, 3:4, sl].broadcast_to([P, 3, wc]),
                    in1=relb[:, :, sl], op=Alu.mult)
                if use_gbias:
                    nc.vector.tensor_copy(out=F12[:, 12:16, sl],
                                          in_=ATT[:, :, sl])
                for ti in range(ta, tb):
                    _tree_reduce(nc, Alu, F12, Fagg, NCH, ti)
                # transpose finished 8-loc blocks as soon as available:
                # slice ends at locs 48 / 80 / 96(of 98) / 112
                fa, fb = [(0, 6), (6, 10), (10, 12), (12, 14)][si]
                nc.sync.dma_start_transpose(
                    out=FaggT[:, fa:fb, :],
                    in_=Fagg[:, 8 * fa:8 * fb, :].rearrange(
                        "p n j -> p (n j)"))

            # 16 -> 32 contraction, two 8-loc blocks per PSUM tile;
            # PSUM->SBUF copies transpose to channel-major and are split
            # across Act and DVE (Act's last copy early so its Sqrt table
            # load hides under the remaining DVE copies)
            for i in range(NBLK // 2):
                seg_ps = ps.tile([P, 512], f32, space="PSUM", tag="seg")
                for k in range(2):
                    b = 2 * i + k
                    nc.tensor.matmul(out=seg_ps[:, 256 * k:256 * (k + 1)],
                                     lhsT=FaggT[:, b, :], rhs=G[:],
                                     start=True, stop=True)
                out_ap = Seg[:, :, 16 * i:16 * (i + 1)]
                in_ap = seg_ps[:].rearrange("p (n c) -> p c n", c=32)
                if i in (0, 2, 4):
                    nc.scalar.activation(out=out_ap, in_=in_ap,
                                         func=Act.Copy)
                else:
                    nc.vector.tensor_copy(out=out_ap, in_=in_ap)

            if use_bout:
                # mean = seg/n needed when bout != 0 (counts no longer cancel)
                nc.vector.tensor_tensor(
                    out=Seg[:, :, :NL], in0=Seg[:, :, :NL],
                    in1=B[:, 4, :].unsqueeze(1).broadcast_to([P, 32, NL]),
                    op=Alu.mult)
                nc.vector.tensor_tensor(
                    out=Seg[:, :, :NL], in0=Seg[:, :, :NL],
                    in1=AUX[:, 0, :].rearrange("p c -> p c 1").broadcast_to(
                        [P, 32, NL]),
                    op=Alu.add)
            # variance + normalize in channel-major layout (all operands
            # packed innermost -> 2x); sqrt per split, silu grouped last
            for (lo, hi) in LN_SPLITS:
                nr = hi - lo
                nc.vector.tensor_tensor(out=sq[:, :, lo:hi],
                                        in0=Seg[:, :, lo:hi],
                                        in1=Seg[:, :, lo:hi], op=Alu.mult)
                cur = 32
                while cur > 2:
                    half = cur // 2
                    nc.vector.tensor_tensor(out=sq[:, :half, lo:hi],
                                            in0=sq[:, :half, lo:hi],
                                            in1=sq[:, half:cur, lo:hi],
                                            op=Alu.add)
                    cur = half
                nc.vector.tensor_tensor(out=var[:, lo:hi],
                                        in0=sq[:, 0, lo:hi],
                                        in1=sq[:, 1, lo:hi], op=Alu.add)
                nc.vector.tensor_tensor(out=var[:, lo:hi],
                                        in0=var[:, lo:hi],
                                        in1=B[:, 3, lo:hi], op=Alu.add)
                nc.scalar.activation(out=std[:, lo:hi], in_=var[:, lo:hi],
                                     func=Act.Sqrt, scale=1.0 / 32)
                with nc.allow_low_precision(reason="bf16 rstd"):
                    nc.vector.reciprocal(out=rstd[:, lo:hi],
                                         in_=std[:, lo:hi])
                nc.vector.tensor_tensor(
                    out=X[:, :, lo:hi], in0=Seg[:, :, lo:hi],
                    in1=rstd[:, lo:hi].unsqueeze(1).broadcast_to(
                        [P, 32, nr]),
                    op=Alu.mult)
                if use_affine:
                    nc.vector.tensor_tensor(
                        out=X[:, :, lo:hi], in0=X[:, :, lo:hi],
                        in1=AUX[:, 1, :].rearrange(
                            "p c -> p c 1").broadcast_to([P, 32, nr]),
                        op=Alu.mult)
                    nc.vector.tensor_tensor(
                        out=X[:, :, lo:hi], in0=X[:, :, lo:hi],
                        in1=AUX[:, 2, :].rearrange(
                            "p c -> p c 1").broadcast_to([P, 32, nr]),
                        op=Alu.add)
            nc.sync.dma_start(out=vr[:], in_=var[:])
            for (lo, hi) in LN_SPLITS:
                nc.scalar.activation(out=Y[:, :, lo:hi], in_=X[:, :, lo:hi],
                                     func=Act.Silu)
                nc.sync.dma_start(out=y[:, :, lo:hi], in_=Y[:, :, lo:hi])
    nc.compile()
    return nc


_CACHE = {}


def _prep(positions, edge_index):
    pos = np.asarray(positions, np.float32)
    row = np.asarray(edge_index[0], np.int64)
    col = np.asarray(edge_index[1], np.int64)
    deg = np.bincount(col, minlength=N_NODES)
    assert deg.max() <= T_D[-1], f"max degree {deg.max()} exceeds {T_D[-1]}"
    order = np.argsort(col, kind="stable")
    col_s, row_s = col[order], row[order]
    starts = np.zeros(N_NODES + 1, np.int64)
    np.cumsum(deg, out=starts[1:])

    in_maps, metas = [], []
    ntier = len(TIERS)
    caps = [T_LOC[t] * P for t in range(ntier)]
    for c in range(N_CORES):
        base = c * NPC
        dloc = deg[base:base + NPC]
        # smallest tier that fits; spill to larger tiers when full
        tier = np.searchsorted(T_D, dloc)
        counts = np.bincount(tier, minlength=ntier)
        for t in range(ntier):
            while counts[t] > caps[t]:
                assert t + 1 < ntier, f"core {c}: tier overflow at {t}"
                n_move = counts[t] - caps[t]
                ids = np.flatnonzero(tier == t)[-n_move:]
                tier[ids] = t + 1
                counts[t] -= n_move
                counts[t + 1] += n_move
        A = np.zeros((P, 3, W), np.float32)
        B = np.zeros((P, 5, NL), np.float32)
        k_of = np.zeros(NPC, np.int64)
        rows_of = np.zeros(NPC, np.int64)
        # per-slot destination index (for dummy fill), then real sources
        dst_of_slot = np.full((P, W), -1, np.int64)
        for ti in range(ntier):
            ids = np.flatnonzero(tier == ti)
            k = np.arange(len(ids))
            k_of[ids] = k
            pp, ll = k // T_LOC[ti], k % T_LOC[ti]
            B[pp, 0:3, T_L0[ti] + ll] = pos[base + ids]
            B[pp, 3, T_L0[ti] + ll] = (
                32.0 * LN_EPS * np.maximum(dloc[ids], 1) ** 2)
            B[pp, 4, T_L0[ti] + ll] = 1.0 / np.maximum(dloc[ids], 1)
            rows_of[ids] = pp * NL + T_L0[ti] + ll
            w_lo = T_W0[ti] + ll * T_D[ti]
            for s in range(T_D[ti]):
                dst_of_slot[pp, w_lo + s] = base + ids
        B[:, 3, :][B[:, 3, :] == 0.0] = 32.0 * LN_EPS  # unused locs
        used = dst_of_slot >= 0
        src_of_slot = dst_of_slot.copy()
        # overwrite the first deg slots of each node with real edge sources
        e0, e1 = starts[base], starts[base + NPC]
        n_loc = (col_s[e0:e1] - base).astype(np.int64)
        slot = np.arange(e0, e1) - starts[col_s[e0:e1]]
        ti_e = tier[n_loc]
        k_e = k_of[n_loc]
        pp_e = k_e // np.array(T_LOC)[ti_e]
        ww_e = (np.array(T_W0[:ntier])[ti_e]
                + (k_e % np.array(T_LOC)[ti_e]) * np.array(T_D)[ti_e] + slot)
        src_of_slot[pp_e, ww_e] = row_s[e0:e1]
        A_src = np.where(used, src_of_slot, 0)
        A[:, 0, :] = pos[A_src, 0] * used
        A[:, 1, :] = pos[A_src, 1] * used
        A[:, 2, :] = pos[A_src, 2] * used
        in_maps.append({"A": A, "B": B})
        metas.append(rows_of)
    return in_maps, metas, (deg, order, col_s, row_s, starts)


_EXEC = {}


def _run_cached(nc, in_maps):
    """bass2jax pjrt run with the jitted executable cached across calls."""
    import jax
    import numpy as _np
    import concourse.mybir as mybir
    from jax.sharding import Mesh, PartitionSpec
    from jax.experimental.shard_map import shard_map
    from concourse import bass2jax as B2J

    key = id(nc)
    if key not in _EXEC:
        B2J.install_neuronx_cc_hook()
        partition_name = (nc.partition_id_tensor.name
                          if nc.partition_id_tensor else None)
        in_names, out_names, out_avals, zero_shapes = [], [], [], []
        for alloc in nc.m.functions[0].allocations:
            if not isinstance(alloc, mybir.MemoryLocationSet):
                continue
            name = alloc.memorylocations[0].name
            if alloc.kind == "ExternalInput":
                if name != partition_name:
                    in_names.append(name)
            elif alloc.kind == "ExternalOutput":
                out_names.append(name)
                shape = tuple(alloc.tensor_shape)
                dtype = mybir.dt.np(alloc.dtype)
                out_avals.append(jax.core.ShapedArray(shape, dtype))
                zero_shapes.append((shape, dtype))
        n_params = len(in_names)
        all_in = list(in_names) + list(out_names)
        if partition_name is not None:
            all_in.append(partition_name)
        donate = tuple(range(n_params, n_params + len(out_names)))

        def _body(*args):
            operands = list(args)
            if partition_name is not None:
                operands.append(B2J.partition_id_tensor())
            return tuple(B2J._bass_exec_p.bind(
                *operands, out_avals=tuple(out_avals), in_names=tuple(all_in),
                out_names=tuple(out_names), lowering_input_output_aliases=(),
                sim_require_finite=True, sim_require_nnan=True, nc=nc))

        devices = jax.devices()[:N_CORES]
        mesh = Mesh(_np.asarray(devices), ("core",))
        specs = (PartitionSpec("core"),) * (n_params + len(out_names))
        fn = jax.jit(
            shard_map(_body, mesh=mesh, in_specs=specs,
                      out_specs=(PartitionSpec("core"),) * len(out_names),
                      check_rep=False),
            donate_argnums=donate, keep_unused=True)
        _EXEC[key] = (fn, in_names, out_names, out_avals, zero_shapes)

    fn, in_names, out_names, out_avals, zero_shapes = _EXEC[key]
    concat_in = [np.concatenate([np.asarray(m[name]) for m in in_maps], axis=0)
                 for name in in_names]
    zeros = [np.zeros((N_CORES * s[0], *s[1:]), d) for s, d in zero_shapes]
    outs = fn(*concat_in, *zeros)
    return [
        {name: np.asarray(outs[i]).reshape(N_CORES, *out_avals[i].shape)[c]
         for i, name in enumerate(out_names)}
        for c in range(N_CORES)
    ]


def _fix_nodes(out, nodes, pos, edge, weights):
    """Exact f32 recompute of the reference math for the given nodes."""
    (deg, order, col_s, row_s, starts) = edge
    (Wq, bq, Wk, bk, Wv, bv, Wout, bout, gamma, beta) = weights
    idx = np.concatenate([np.arange(starts[n], starts[n + 1]) for n in nodes]
                         ) if len(nodes) else np.zeros(0, np.int64)
    remap = {n: i for i, n in enumerate(nodes)}
    seg = np.zeros((len(nodes), HEADS * HIDDEN), np.float32)
    if len(idx):
        rows, cols = row_s[idx], col_s[idx]
        rel = pos[rows] - pos[cols]
        q = (rel @ Wq + bq).reshape(-1, HEADS, HIDDEN)
        k = (rel @ Wk + bk).reshape(-1, HEADS, HIDDEN)
        v = (rel @ Wv + bv).reshape(-1, HEADS, HIDDEN)
        sc = (q * k).sum(-1) / np.sqrt(np.float32(HIDDEN))
        sc -= sc.max(-1, keepdims=True)
        a = np.exp(sc)
        a /= a.sum(-1, keepdims=True)
        wv = (a[..., None] * v).reshape(-1, HEADS * HIDDEN)
        np.add.at(seg, np.array([remap[c] for c in cols]), wv)
    mean = seg / np.maximum(deg[nodes], 1)[:, None]
    o = mean @ Wout + bout
    mu = o.mean(-1, keepdims=True)
    va = o.var(-1, keepdims=True)
    o = (o - mu) / np.sqrt(va + LN_EPS) * gamma + beta
    out[nodes] = o / (1.0 + np.exp(-o))


def kernel(positions, edge_index, Wq, bq, Wk, bk, Wv, bv, Wout, bout,
           gamma, beta):
    import ml_dtypes

    positions = np.asarray(positions, np.float32)
    args = [np.asarray(x, np.float32)
            for x in (Wq, bq, Wk, bk, Wv, bv, Wout)]
    bq_, bk_ = args[1], args[3]
    assert not np.any(bq_) and not np.any(bk_), \
        "nonzero q/k biases not folded in this kernel"
    bout = np.asarray(bout, np.float32)
    gamma = np.asarray(gamma, np.float32)
    beta = np.asarray(beta, np.float32)
    Cd, G16 = _fold_weights(*args)
    use_bout = bool(np.any(bout != 0))
    use_affine = bool(np.any(gamma != 1) or np.any(beta != 0))
    use_gbias = bool(np.any(G16[12:16, :] != 0))

    key = (use_bout, use_affine, use_gbias)
    if key not in _CACHE:
        _CACHE[key] = _build_bass(use_bout, use_affine, use_gbias)
    nc = _CACHE[key]

    # centered output projection: LN is shift-invariant
    Gc = G16 - G16.mean(axis=1, keepdims=True)
    if not use_gbias:
        Gc[12:16, :] = 0.0
    Gblk = np.zeros((P, 256), np.float32)
    for loc in range(8):
        Gblk[16 * loc:16 * loc + 16, 32 * loc:32 * loc + 32] = Gc
    Gblk = Gblk.astype(ml_dtypes.bfloat16)
    # block-diagonal delta-score weights: 16 slots x (8 mono -> 3 heads)
    CdBlk = np.zeros((P, 48), np.float32)
    for s in range(16):
        CdBlk[8 * s:8 * s + 6, 3 * s:3 * s + 3] = Cd
    CdBlk = CdBlk.astype(ml_dtypes.bfloat16)

    in_maps, metas, edge = _prep(positions, edge_index)
    deg = edge[0]
    aux = np.zeros((P, 3, 32), np.float32)
    aux[:, 0, :] = bout - bout.mean()
    aux[:, 1, :] = gamma
    aux[:, 2, :] = beta
    for m in in_maps:
        m["G"] = Gblk
        m["CD"] = CdBlk
        m["AUX"] = aux
        if use_bout:
            # plain eps when the mean is materialized
            m["B"][:, 3, :] = 32.0 * LN_EPS
    res = _run_cached(nc, in_maps)

    out = np.empty((N_NODES, 32), np.float32)
    var_mean = np.empty(N_NODES, np.float32)
    n2 = np.maximum(deg, 1).astype(np.float32) ** 2
    for c in range(N_CORES):
        base = c * NPC
        yv = np.asarray(res[c]["y"]).astype(np.float32)   # [P, 32, NL]
        out[base:base + NPC] = yv.transpose(0, 2, 1).reshape(
            P * NL, 32)[metas[c]]
        vv = np.asarray(res[c]["vr"]).reshape(P * NL)[metas[c]]
        var_mean[base:base + NPC] = vv
    if not use_bout:
        var_mean = var_mean / (32.0 * n2) - LN_EPS
    else:
        var_mean = var_mean / 32.0 - LN_EPS
    # recompute ill-conditioned nodes (LN variance amplifies bf16 rounding)
    bad = np.flatnonzero(var_mean < VAR_TAU)
    if len(bad):
        _fix_nodes(out, bad, positions, edge,
                   (*args, bout, gamma, beta))
    return out
